# revision 1
# baseline (speedup 1.0000x reference)
"""ConsistencyLoss kernel for 8 Trainium2 NeuronCores.

Math (per reference):
  For view1: sim = cos_sim_pairwise(y1, z2) [B,N,N]; mask from grid distances;
  loss_v = sum(sim*mask)/sum(mask); out = -(loss_1 + loss_2), N = 28*28 = 784.

Strategy: data-parallel over batch (8 batches/core x 8 cores).
  Host prep (cheap O(B*C*N) work):
    - The grids produced by the reference are exactly separable:
      grid[b,0,i,j] depends only on i, grid[b,1,i,j] only on j.  So the
      pairwise squared distance D2[n,m] = Dy2[i(n),i'(m)] + Dx2[j(n),j'(m)]
      with two tiny [28,28] tables per batch.
    - The mask is a narrow diagonal band: for each 128-row tile of n, every
      masked m lies in a WW=12-image-row window of m whose start the host
      computes from Dy2 (window provably covers all masked pairs).  The
      device only evaluates the [128, 336] window instead of [128, 784].
    - Norms of all four feature tensors; 1/norm of the z-side is folded into
      the z features; 1/norm of the y-side is applied on-device to tiny
      per-tile accumulators (O(N) not O(N^2)).
    - Mask counts (denominators) are computed on host with bit-identical
      fp32 arithmetic to the device mask test.
  Device per batch (n tiled as 6x128+16 partitions):
    - PE: num = y^T @ z_hat windowed (float32r inputs, fp32 PSUM accumulate);
      the moving-operand window offset is a runtime value loaded into a PE
      register (bass.ds dynamic slice).
    - DVE: assemble windowed D2 tiles [128,336] from broadcast APs; fused
      (D2 <= t^2) * num with per-partition accumulation
      (scalar_tensor_tensor); rna-weighted reduction of [128,7] accumulators.
    - Final: partition-reduce via ones-matmul -> [1,2] per-core output.
  Host finish: sum the 8 cores' masked sums, divide by host counts.
"""

import sys

sys.path.insert(0, "/opt/trn_rl_repo")

import numpy as np

import concourse.bass as bass
import concourse.mybir as mybir
import concourse.tile as tile
from concourse import bacc
from concourse.bass import broadcast_tensor_aps
from concourse.bass_utils import run_bass_kernel_spmd

B, C, H, W = 64, 256, 28, 28
N = H * W  # 784
NCORES = 8
BPC = B // NCORES  # batches per core
NT = 7  # n tiles: 6 full 128-partition tiles + one 16-partition tile
NPAD = NT * 128  # 896
THR = 0.7
WW = 12  # window rows (i') per n-tile
WWC = WW * 28  # 336 window columns in m

F32 = mybir.dt.float32
F16 = mybir.dt.float16
F32R = mybir.dt.float32r
I32 = mybir.dt.int32
ALU = mybir.AluOpType
ENG = mybir.EngineType

_COMPILED = {}


def _build_nc():
    nc = bacc.Bacc("TRN2", debug=False, num_devices=NCORES)

    ins = {}
    for nm in ("ay1", "ay2", "bz2", "bz1"):
        ins[nm] = nc.dram_tensor(nm, [BPC, 128, 2, N], F32R, kind="ExternalInput")
    ins["dyw"] = nc.dram_tensor("dyw", [BPC, NT, 128, WW], F32, kind="ExternalInput")
    ins["dx2p"] = nc.dram_tensor("dx2p", [BPC, NPAD, 28], F32, kind="ExternalInput")
    ins["thr"] = nc.dram_tensor("thr", [BPC, 128, 2], F32, kind="ExternalInput")
    ins["rna"] = nc.dram_tensor("rna", [BPC, 128, 2, NT], F32, kind="ExternalInput")
    ins["woff"] = nc.dram_tensor("woff", [BPC, 1, NT], I32, kind="ExternalInput")
    out = nc.dram_tensor("out", [1, 2], F32, kind="ExternalOutput")

    with tile.TileContext(nc) as tc:
        with (
            tc.tile_pool(name="feat", bufs=2) as feat_pool,
            tc.tile_pool(name="dyx", bufs=2) as dyx_pool,
            tc.tile_pool(name="d2", bufs=3) as d2_pool,
            tc.tile_pool(name="scr", bufs=3) as scr_pool,
            tc.tile_pool(name="ms", bufs=2) as ms_pool,
            tc.tile_pool(name="small", bufs=2) as sm_pool,
            tc.tile_pool(name="accum", bufs=1) as acc_pool,
            tc.tile_pool(name="psum", bufs=6, space="PSUM") as psum_pool,
            tc.tile_pool(name="psumf", bufs=1, space="PSUM") as psumf_pool,
        ):
            stot = acc_pool.tile([128, 2, BPC], F32)
            ones_col = acc_pool.tile([128, 1], F32)
            nc.vector.memset(ones_col[:, :], 1.0)

            for b in range(BPC):
                feats = {}
                for nm in ("ay1", "ay2", "bz2", "bz1"):
                    t = feat_pool.tile([128, 2, N], F32R, tag=nm)
                    nc.sync.dma_start(t[:, :, :], ins[nm][b])
                    feats[nm] = t
                dyw_t = dyx_pool.tile([128, NT, WW], F32, tag="dy")
                nc.sync.dma_start(
                    dyw_t[:, :, :], ins["dyw"][b].rearrange("k p w -> p k w")
                )
                dx_t = dyx_pool.tile([128, NT, 28], F32, tag="dx")
                nc.sync.dma_start(
                    dx_t[:, :, :], ins["dx2p"][b].rearrange("(k p) i -> p k i", p=128)
                )
                thr_t = sm_pool.tile([128, 2], F32, tag="thr")
                nc.sync.dma_start(thr_t[:, :], ins["thr"][b])
                rna_t = sm_pool.tile([128, 2, NT], F32, tag="rna")
                nc.sync.dma_start(rna_t[:, :, :], ins["rna"][b])
                woff_t = sm_pool.tile([1, NT], I32, tag="woff")
                nc.sync.dma_start(woff_t[:, :], ins["woff"][b])

                ms = []
                for v in (0, 1):
                    m = ms_pool.tile([128, NT], F32, tag=f"ms{v}")
                    nc.vector.memset(m[:, :], 0.0)
                    ms.append(m)

                for k in range(NT):
                    p = 128 if k < 6 else N - 6 * 128
                    d2 = d2_pool.tile([128, WWC], F32, tag="d2")
                    i0, i1 = broadcast_tensor_aps(
                        dyw_t[:, k, :, None], dx_t[:, k, None, :]
                    )
                    nc.vector.tensor_tensor(
                        d2[:, :].rearrange("q (a c) -> q a c", a=WW), i0, i1, ALU.add
                    )
                    nums = []
                    reg = nc.alloc_registers(
                        name=f"w_{b}_{k}", engines=(ENG.PE,)
                    )
                    nc.tensor.load(reg, woff_t[0:1, k : k + 1])
                    wv = nc.snap(reg, donate=True, min_val=0,
                                 max_val=(28 - WW) * 28)
                    for v, (a_nm, b_nm) in enumerate(
                        (("ay1", "bz2"), ("ay2", "bz1"))
                    ):
                        num = psum_pool.tile([128, WWC], F32, tag="num")
                        a_t, b_t = feats[a_nm], feats[b_nm]
                        for cc in (0, 1):
                            nc.tensor.matmul(
                                num[0:p, :],
                                a_t[:, cc, k * 128 : k * 128 + p],
                                b_t[:, cc, bass.ds(wv, WWC)],
                                start=(cc == 0),
                                stop=(cc == 1),
                            )
                        nums.append(num)
                    for v in (0, 1):
                        scr = scr_pool.tile([128, WWC], F32, tag="scr")
                        nc.vector.scalar_tensor_tensor(
                            out=scr[0:p, :],
                            in0=d2[0:p, :],
                            scalar=thr_t[0:p, v : v + 1],
                            in1=nums[v][0:p, :],
                            op0=ALU.is_le,
                            op1=ALU.mult,
                            accum_out=ms[v][0:p, k : k + 1],
                        )

                for v in (0, 1):
                    wscr = scr_pool.tile([128, NT], F32, tag="wscr")
                    nc.vector.scalar_tensor_tensor(
                        out=wscr[:, :],
                        in0=ms[v][:, :],
                        scalar=1.0,
                        in1=rna_t[:, v, :],
                        op0=ALU.mult,
                        op1=ALU.mult,
                        accum_out=stot[:, v, b : b + 1],
                    )

            sfin = acc_pool.tile([128, 2], F32)
            nc.vector.reduce_sum(sfin[:, :], stot[:, :, :], axis=mybir.AxisListType.X)
            ps = psumf_pool.tile([1, 2], F32)
            nc.tensor.matmul(ps[:, :], ones_col[:, :], sfin[:, :], start=True, stop=True)
            out_s = acc_pool.tile([1, 2], F32)
            nc.vector.tensor_copy(out_s[:, :], ps[:, :])
            nc.sync.dma_start(out[:, :], out_s[:, :])

    nc.compile()
    return nc


def _get_nc():
    if "nc" not in _COMPILED:
        _COMPILED["nc"] = _build_nc()
    return _COMPILED["nc"]


def _prep_host(y1, y2, z1, z2, view1_grid, view2_grid):
    """Host-side prep: separable distance tables, norms, counts, shards."""
    y1f = y1.reshape(B, C, N)
    y2f = y2.reshape(B, C, N)
    z1f = z1.reshape(B, C, N)
    z2f = z2.reshape(B, C, N)

    # --- separable grid tables ------------------------------------------
    g1y = view1_grid[:, 0, :, 0]  # [B, 28] rows (y coordinate per i)
    g1x = view1_grid[:, 1, 0, :]  # [B, 28] cols (x coordinate per j)
    g2y = view2_grid[:, 0, :, 0]
    g2x = view2_grid[:, 1, 0, :]
    if not (
        np.array_equal(view1_grid[:, 0], np.broadcast_to(g1y[:, :, None], (B, H, W)))
        and np.array_equal(view1_grid[:, 1], np.broadcast_to(g1x[:, None, :], (B, H, W)))
        and np.array_equal(view2_grid[:, 0], np.broadcast_to(g2y[:, :, None], (B, H, W)))
        and np.array_equal(view2_grid[:, 1], np.broadcast_to(g2x[:, None, :], (B, H, W)))
    ):
        raise RuntimeError("grids are not separable; unsupported input")

    dy = g1y[:, :, None] - g2y[:, None, :]  # fp32 [B,28,28]
    dx = g1x[:, :, None] - g2x[:, None, :]
    dy2 = dy * dy
    dx2 = dx * dx

    v1bin = np.linalg.norm(
        view1_grid[..., 1, 1] - view1_grid[..., 0, 0], axis=-1
    )  # [B]
    v2bin = np.linalg.norm(view2_grid[..., 1, 1] - view2_grid[..., 0, 0], axis=-1)
    t2 = np.empty((B, 2), np.float32)
    t2[:, 0] = ((THR * v1bin.astype(np.float64)) ** 2).astype(np.float32)
    t2[:, 1] = ((THR * v2bin.astype(np.float64)) ** 2).astype(np.float32)

    # --- per-(batch, tile) windows of valid i' --------------------------
    tmax2 = np.maximum(t2[:, 0], t2[:, 1]).astype(np.float64) * (1 + 1e-6)  # [B]
    w0 = np.zeros((B, NT), np.int32)
    dyw = np.zeros((B, NT, 128, WW), np.float32)
    iidx_all = np.minimum(np.arange(NPAD), N - 1) // 28  # [896]
    for k in range(NT):
        p = 128 if k < 6 else N - 6 * 128
        n0 = 128 * k
        i_lo = n0 // 28
        i_hi = (n0 + p - 1) // 28
        sub_min = dy2[:, i_lo : i_hi + 1, :].min(axis=1)  # [B, 28]
        valid = sub_min <= tmax2[:, None]  # [B, 28]
        any_valid = valid.any(axis=1)
        first = np.argmax(valid, axis=1)
        last = 27 - np.argmax(valid[:, ::-1], axis=1)
        width = np.where(any_valid, last - first + 1, 1)
        if (width > WW).any():
            raise RuntimeError("mask window exceeds WW; unsupported input")
        w0k = np.minimum(np.where(any_valid, first, 0), 28 - WW).astype(np.int32)
        w0[:, k] = w0k
        iidx = iidx_all[n0 : n0 + 128]  # [128]
        cols = w0k[:, None] + np.arange(WW)[None, :]  # [B, WW]
        dyw[:, k] = dy2[
            np.arange(B)[:, None, None], iidx[None, :, None], cols[:, None, :]
        ]
    woff = (w0 * 28).astype(np.int32).reshape(B, 1, NT)

    dx2p = np.zeros((B, NPAD, 28), np.float32)
    dx2p[:, :N] = np.tile(dx2, (1, H, 1))  # row j(n) = n % 28

    # --- mask counts (bit-identical fp32 add + compare as device) -------
    counts = np.zeros(2, np.int64)
    for b in range(B):
        d2b = dy2[b][:, None, :, None] + dx2[b][None, :, None, :]  # fp32
        counts[0] += int((d2b <= t2[b, 0]).sum())
        counts[1] += int((d2b <= t2[b, 1]).sum())

    # --- norms ----------------------------------------------------------
    def rnorm(a):
        n = np.sqrt(np.einsum("bcn,bcn->bn", a, a, dtype=np.float32))
        return 1.0 / np.maximum(n, np.float32(1e-7))

    rna1 = rnorm(y1f)  # y-side view1  [B, N]
    rna2 = rnorm(y2f)  # y-side view2
    rnb1 = rnorm(z2f)  # z-side view1
    rnb2 = rnorm(z1f)  # z-side view2

    def pack_feat(a):
        # [B, C, N] fp -> [B, 128, 2, N] fp32 (float32r on device)
        return np.ascontiguousarray(
            a.reshape(B, 2, 128, N).transpose(0, 2, 1, 3).astype(np.float32)
        )

    ay1 = pack_feat(y1f)
    ay2 = pack_feat(y2f)
    bz2 = pack_feat(z2f * rnb1[:, None, :])
    bz1 = pack_feat(z1f * rnb2[:, None, :])

    # y-side reciprocal norms in [B, 128, 2, NT] layout (zero padded)
    rna = np.zeros((B, 128, 2, NT), np.float32)
    pad1 = np.zeros((B, NPAD), np.float32)
    pad2 = np.zeros((B, NPAD), np.float32)
    pad1[:, :N] = rna1
    pad2[:, :N] = rna2
    rna[:, :, 0, :] = pad1.reshape(B, NT, 128).transpose(0, 2, 1)
    rna[:, :, 1, :] = pad2.reshape(B, NT, 128).transpose(0, 2, 1)

    thr = np.ascontiguousarray(np.broadcast_to(t2[:, None, :], (B, 128, 2)))

    in_maps = []
    for c in range(NCORES):
        s = slice(c * BPC, (c + 1) * BPC)
        in_maps.append(
            {
                "ay1": ay1[s],
                "ay2": ay2[s],
                "bz2": bz2[s],
                "bz1": bz1[s],
                "dyw": np.ascontiguousarray(dyw[s]),
                "dx2p": np.ascontiguousarray(dx2p[s]),
                "thr": thr[s],
                "rna": np.ascontiguousarray(rna[s]),
                "woff": np.ascontiguousarray(woff[s]),
            }
        )
    return in_maps, counts


def kernel(y1, y2, z1, z2, view1_grid, view2_grid):
    y1 = np.asarray(y1, np.float32)
    y2 = np.asarray(y2, np.float32)
    z1 = np.asarray(z1, np.float32)
    z2 = np.asarray(z2, np.float32)
    view1_grid = np.asarray(view1_grid, np.float32)
    view2_grid = np.asarray(view2_grid, np.float32)

    in_maps, counts = _prep_host(y1, y2, z1, z2, view1_grid, view2_grid)
    nc = _get_nc()
    res = run_bass_kernel_spmd(nc, in_maps, core_ids=list(range(NCORES)))
    s = np.zeros(2, np.float64)
    for i in range(NCORES):
        s += res.results[i]["out"][0].astype(np.float64)
    loss = -(
        np.float32(s[0]) / np.float32(counts[0])
        + np.float32(s[1]) / np.float32(counts[1])
    )
    return np.array(loss, dtype=np.float32)



# revision 2
# speedup vs baseline: 1.4724x; 1.4724x over previous
"""ConsistencyLoss kernel for 8 Trainium2 NeuronCores.

Math (per reference):
  For view1: sim = cos_sim_pairwise(y1, z2) [B,N,N]; mask from grid distances;
  loss_v = sum(sim*mask)/sum(mask); out = -(loss_1 + loss_2), N = 28*28 = 784.

Strategy: data-parallel over batch (8 batches/core x 8 cores).
  Host prep (cheap O(B*C*N) numpy):
    - The reference grids are separable: grid[b,0,i,j] depends only on i,
      grid[b,1,i,j] only on j.  Pairwise squared distance
      D2[n,m] = Dy2[i(n),i'(m)] + Dx2[j(n),j'(m)] from two [28,28] tables.
    - n is tiled in 7 groups of 4 image rows (112 partitions, aligned to the
      28-col image width).  For each tile the masked i' band spans at most
      WW=6 image rows whose start the host computes; the device evaluates
      only the [112, 168] window instead of [112, 784].
    - Features are shipped as float16 (halves HBM traffic; matmul runs at
      1 cycle/row).  1/norm of the z-side is folded into the z features
      (fp32 math, then f16 cast); 1/norm of the y-side is applied on-device
      to tiny per-tile accumulators.
    - Mask counts (denominators) are computed on host with bit-identical
      fp32 arithmetic to the device mask test.
  Device per batch:
    - PE: num = y^T @ z_hat windowed (f16 inputs, fp32 PSUM accumulate);
      window offset is a runtime PE register (bass.ds dynamic slice).
    - GpSimd: assemble windowed D2 tiles [112,168] from broadcast APs.
    - DVE: fused (D2 <= t^2) * num with per-partition accumulation
      (scalar_tensor_tensor, scratch to PSUM); rna-weighted reduction of
      the [112,7] accumulators.
    - Final: partition-reduce via ones-matmul -> [1,2] per-core output.
  Host finish: sum the 8 cores' masked sums, divide by host counts.
"""

import sys

sys.path.insert(0, "/opt/trn_rl_repo")

import numpy as np

import concourse.bass as bass
import concourse.mybir as mybir
import concourse.tile as tile
from concourse import bacc
from concourse.bass import broadcast_tensor_aps
from concourse.bass_utils import run_bass_kernel_spmd

B, C, H, W = 64, 256, 28, 28
N = H * W  # 784
NCORES = 8
BPC = B // NCORES  # batches per core
NT = 7  # n tiles: 7 groups of 4 image rows
P = 112  # partitions per tile (4 image rows)
THR = 0.7
WW = 6  # window rows (i') per n-tile (verified on host against inputs)
WWC = WW * 28  # 168 window columns in m

F32 = mybir.dt.float32
F16 = mybir.dt.float16
I32 = mybir.dt.int32
ALU = mybir.AluOpType
ENG = mybir.EngineType

_COMPILED = {}


def _build_nc():
    nc = bacc.Bacc("TRN2", debug=False, num_devices=NCORES)

    # feature order along dim1: y1, y2, z2_hat, z1_hat
    feat = nc.dram_tensor("feat", [BPC, 128, 4, 2, N], F16, kind="ExternalInput")
    dyw_i = nc.dram_tensor("dyw", [BPC, P, NT, WW], F32, kind="ExternalInput")
    dx_i = nc.dram_tensor("dx", [BPC, P, 28], F32, kind="ExternalInput")
    thr_i = nc.dram_tensor("thr", [BPC, P, 2], F32, kind="ExternalInput")
    rna_i = nc.dram_tensor("rna", [BPC, P, 2, NT], F32, kind="ExternalInput")
    woff_i = nc.dram_tensor("woff", [BPC, 1, NT], I32, kind="ExternalInput")
    out = nc.dram_tensor("out", [1, 2], F32, kind="ExternalOutput")

    with tile.TileContext(nc) as tc:
        with (
            tc.tile_pool(name="feat", bufs=2) as feat_pool,
            tc.tile_pool(name="dyx", bufs=2) as dyx_pool,
            tc.tile_pool(name="d2", bufs=3) as d2_pool,
            tc.tile_pool(name="ms", bufs=2) as ms_pool,
            tc.tile_pool(name="small", bufs=2) as sm_pool,
            tc.tile_pool(name="accum", bufs=1) as acc_pool,
            tc.tile_pool(name="psum", bufs=4, space="PSUM") as psum_pool,
            tc.tile_pool(name="pscr", bufs=3, space="PSUM") as pscr_pool,
            tc.tile_pool(name="psumf", bufs=1, space="PSUM") as psumf_pool,
        ):
            stot = acc_pool.tile([P, 2, BPC], F32)
            ones_col = acc_pool.tile([P, 1], F32)
            nc.vector.memset(ones_col[:, :], 1.0)

            for b in range(BPC):
                ft = feat_pool.tile([128, 4, 2, N], F16, tag="feat")
                nc.sync.dma_start(ft[:, :, :, :], feat[b])
                dyw_t = dyx_pool.tile([P, NT, WW], F32, tag="dy")
                nc.sync.dma_start(dyw_t[:, :, :], dyw_i[b])
                dx_t = dyx_pool.tile([P, 28], F32, tag="dx")
                nc.sync.dma_start(dx_t[:, :], dx_i[b])
                thr_t = sm_pool.tile([P, 2], F32, tag="thr")
                nc.sync.dma_start(thr_t[:, :], thr_i[b])
                rna_t = sm_pool.tile([P, 2, NT], F32, tag="rna")
                nc.sync.dma_start(rna_t[:, :, :], rna_i[b])
                woff_t = sm_pool.tile([1, NT], I32, tag="woff")
                nc.sync.dma_start(woff_t[:, :], woff_i[b])

                ms = []
                for v in (0, 1):
                    m = ms_pool.tile([P, NT], F32, tag=f"ms{v}")
                    nc.vector.memset(m[:, :], 0.0)
                    ms.append(m)

                for k in range(NT):
                    d2 = d2_pool.tile([P, WWC], F32, tag="d2")
                    i0, i1 = broadcast_tensor_aps(
                        dyw_t[:, k, :, None], dx_t[:, None, :]
                    )
                    nc.gpsimd.tensor_tensor(
                        d2[:, :].rearrange("q (a c) -> q a c", a=WW), i0, i1, ALU.add
                    )
                    reg = nc.alloc_registers(name=f"w_{b}_{k}", engines=(ENG.PE,))
                    nc.tensor.load(reg, woff_t[0:1, k : k + 1])
                    wv = nc.snap(reg, donate=True, min_val=0, max_val=(28 - WW) * 28)
                    nums = []
                    for v, (ai, bi) in enumerate(((0, 2), (1, 3))):
                        num = psum_pool.tile([P, WWC], F32, tag="num")
                        for cc in (0, 1):
                            nc.tensor.matmul(
                                num[:, :],
                                ft[:, ai, cc, k * P : (k + 1) * P],
                                ft[:, bi, cc, bass.ds(wv, WWC)],
                                start=(cc == 0),
                                stop=(cc == 1),
                            )
                        nums.append(num)
                    for v in (0, 1):
                        scr = pscr_pool.tile([P, WWC], F32, tag="scr")
                        nc.vector.scalar_tensor_tensor(
                            out=scr[:, :],
                            in0=d2[:, :],
                            scalar=thr_t[:, v : v + 1],
                            in1=nums[v][:, :],
                            op0=ALU.is_le,
                            op1=ALU.mult,
                            accum_out=ms[v][:, k : k + 1],
                        )

                for v in (0, 1):
                    wscr = sm_pool.tile([P, NT], F32, tag="wscr")
                    nc.vector.scalar_tensor_tensor(
                        out=wscr[:, :],
                        in0=ms[v][:, :],
                        scalar=1.0,
                        in1=rna_t[:, v, :],
                        op0=ALU.mult,
                        op1=ALU.mult,
                        accum_out=stot[:, v, b : b + 1],
                    )

            sfin = acc_pool.tile([P, 2], F32)
            nc.vector.reduce_sum(sfin[:, :], stot[:, :, :], axis=mybir.AxisListType.X)
            ps = psumf_pool.tile([1, 2], F32)
            nc.tensor.matmul(ps[:, :], ones_col[:, :], sfin[:, :], start=True, stop=True)
            out_s = acc_pool.tile([1, 2], F32)
            nc.vector.tensor_copy(out_s[:, :], ps[:, :])
            nc.sync.dma_start(out[:, :], out_s[:, :])

    nc.compile()
    return nc


def _get_nc():
    if "nc" not in _COMPILED:
        _COMPILED["nc"] = _build_nc()
    return _COMPILED["nc"]


def _prep_host(y1, y2, z1, z2, view1_grid, view2_grid):
    """Host-side prep: separable distance tables, norms, counts, shards."""
    y1f = y1.reshape(B, C, N)
    y2f = y2.reshape(B, C, N)
    z1f = z1.reshape(B, C, N)
    z2f = z2.reshape(B, C, N)

    # --- separable grid tables ------------------------------------------
    g1y = view1_grid[:, 0, :, 0]  # [B, 28] rows (y coordinate per i)
    g1x = view1_grid[:, 1, 0, :]  # [B, 28] cols (x coordinate per j)
    g2y = view2_grid[:, 0, :, 0]
    g2x = view2_grid[:, 1, 0, :]
    if not (
        np.array_equal(view1_grid[:, 0], np.broadcast_to(g1y[:, :, None], (B, H, W)))
        and np.array_equal(view1_grid[:, 1], np.broadcast_to(g1x[:, None, :], (B, H, W)))
        and np.array_equal(view2_grid[:, 0], np.broadcast_to(g2y[:, :, None], (B, H, W)))
        and np.array_equal(view2_grid[:, 1], np.broadcast_to(g2x[:, None, :], (B, H, W)))
    ):
        raise RuntimeError("grids are not separable; unsupported input")

    dy = g1y[:, :, None] - g2y[:, None, :]  # fp32 [B,28,28]
    dx = g1x[:, :, None] - g2x[:, None, :]
    dy2 = dy * dy
    dx2 = dx * dx

    v1bin = np.linalg.norm(
        view1_grid[..., 1, 1] - view1_grid[..., 0, 0], axis=-1
    )  # [B]
    v2bin = np.linalg.norm(view2_grid[..., 1, 1] - view2_grid[..., 0, 0], axis=-1)
    t2 = np.empty((B, 2), np.float32)
    t2[:, 0] = ((THR * v1bin.astype(np.float64)) ** 2).astype(np.float32)
    t2[:, 1] = ((THR * v2bin.astype(np.float64)) ** 2).astype(np.float32)

    # --- per-(batch, tile) windows of valid i' --------------------------
    # Tiles are 4 aligned image rows (112 partitions).  A masked pair has
    # dy2 <= d2/(1-2^-24) <= t2*(1+1.2e-7) < tmax2, so the [first,last]
    # band below covers every masked i'.
    tmax2 = np.maximum(t2[:, 0], t2[:, 1]).astype(np.float64) * (1 + 1e-6)  # [B]
    w0 = np.zeros((B, NT), np.int32)
    dyw = np.zeros((B, P, NT, WW), np.float32)
    iidx = np.arange(P) // 28  # [112] image row within tile
    for k in range(NT):
        sub_min = dy2[:, 4 * k : 4 * k + 4, :].min(axis=1)  # [B, 28]
        valid = sub_min <= tmax2[:, None]  # [B, 28]
        any_valid = valid.any(axis=1)
        first = np.argmax(valid, axis=1)
        last = 27 - np.argmax(valid[:, ::-1], axis=1)
        width = np.where(any_valid, last - first + 1, 1)
        if (width > WW).any():
            raise RuntimeError("mask window exceeds WW; unsupported input")
        w0k = np.minimum(np.where(any_valid, first, 0), 28 - WW).astype(np.int32)
        w0[:, k] = w0k
        cols = w0k[:, None] + np.arange(WW)[None, :]  # [B, WW]
        dyw[:, :, k, :] = dy2[
            np.arange(B)[:, None, None],
            (4 * k + iidx)[None, :, None],
            cols[:, None, :],
        ]
    woff = (w0 * 28).astype(np.int32).reshape(B, 1, NT)

    dxt = np.ascontiguousarray(
        np.broadcast_to(dx2[:, None, :, :], (B, 4, 28, 28)).reshape(B, P, 28)
    )

    # --- mask counts (bit-identical fp32 add + compare as device) -------
    counts = np.zeros(2, np.int64)
    for b in range(B):
        d2b = dy2[b][:, None, :, None] + dx2[b][None, :, None, :]  # fp32
        counts[0] += int((d2b <= t2[b, 0]).sum())
        counts[1] += int((d2b <= t2[b, 1]).sum())

    # --- norms ----------------------------------------------------------
    def rnorm(a):
        n = np.sqrt(np.einsum("bcn,bcn->bn", a, a, dtype=np.float32))
        return 1.0 / np.maximum(n, np.float32(1e-7))

    rna1 = rnorm(y1f)  # y-side view1  [B, N]
    rna2 = rnorm(y2f)  # y-side view2
    rnb1 = rnorm(z2f)  # z-side view1
    rnb2 = rnorm(z1f)  # z-side view2

    def pack_feat(a):
        # [B, C, N] fp32 -> [B, 128, 2, N] f16
        return a.reshape(B, 2, 128, N).transpose(0, 2, 1, 3).astype(np.float16)

    featall = np.stack(
        [
            pack_feat(y1f),
            pack_feat(y2f),
            pack_feat(z2f * rnb1[:, None, :]),
            pack_feat(z1f * rnb2[:, None, :]),
        ],
        axis=2,
    )  # [B, 128, 4, 2, N]

    # y-side reciprocal norms in [B, P, 2, NT] layout
    rna = np.empty((B, P, 2, NT), np.float32)
    rna[:, :, 0, :] = rna1.reshape(B, NT, P).transpose(0, 2, 1)
    rna[:, :, 1, :] = rna2.reshape(B, NT, P).transpose(0, 2, 1)

    thr = np.ascontiguousarray(np.broadcast_to(t2[:, None, :], (B, P, 2)))

    in_maps = []
    for c in range(NCORES):
        s = slice(c * BPC, (c + 1) * BPC)
        in_maps.append(
            {
                "feat": np.ascontiguousarray(featall[s]),
                "dyw": np.ascontiguousarray(dyw[s]),
                "dx": dxt[s],
                "thr": thr[s],
                "rna": np.ascontiguousarray(rna[s]),
                "woff": np.ascontiguousarray(woff[s]),
            }
        )
    return in_maps, counts


def kernel(y1, y2, z1, z2, view1_grid, view2_grid):
    y1 = np.asarray(y1, np.float32)
    y2 = np.asarray(y2, np.float32)
    z1 = np.asarray(z1, np.float32)
    z2 = np.asarray(z2, np.float32)
    view1_grid = np.asarray(view1_grid, np.float32)
    view2_grid = np.asarray(view2_grid, np.float32)

    in_maps, counts = _prep_host(y1, y2, z1, z2, view1_grid, view2_grid)
    nc = _get_nc()
    res = run_bass_kernel_spmd(nc, in_maps, core_ids=list(range(NCORES)))
    s = np.zeros(2, np.float64)
    for i in range(NCORES):
        s += res.results[i]["out"][0].astype(np.float64)
    loss = -(
        np.float32(s[0]) / np.float32(counts[0])
        + np.float32(s[1]) / np.float32(counts[1])
    )
    return np.array(loss, dtype=np.float32)


# revision 4
# speedup vs baseline: 1.6241x; 1.1031x over previous
"""ConsistencyLoss kernel for 8 Trainium2 NeuronCores.

Math (per reference):
  For view1: sim = cos_sim_pairwise(y1, z2) [B,N,N]; mask from grid distances;
  loss_v = sum(sim*mask)/sum(mask); out = -(loss_1 + loss_2), N = 28*28 = 784.

Strategy: data-parallel over batch (8 batches/core x 8 cores).
  Host prep (cheap O(B*C*N) numpy):
    - The reference grids are separable: grid[b,0,i,j] depends only on i,
      grid[b,1,i,j] only on j.  Pairwise squared distance
      D2[n,m] = Dy2[i(n),i'(m)] + Dx2[j(n),j'(m)] from two [28,28] tables.
    - n is tiled in 7 groups of 4 image rows (112 partitions, aligned to the
      28-col image width).  For each tile the masked i' band spans at most
      WW=6 image rows whose start the host computes; the device evaluates
      only the [112, 168] window instead of [112, 784].
    - BOTH feature sides are normalized on host (fp32) and shipped as f16,
      so the windowed matmul directly produces cosine sims and the masked
      sums accumulate freely across tiles.  y-side is padded to 800 cols so
      every stationary load is a full 128 columns (enables FWL).
    - Mask counts (denominators) are computed on host with bit-identical
      fp32 arithmetic to the device mask test.
  Device per batch:
    - PE: sim = y_hat^T @ z_hat windowed (f16, fp32 PSUM accumulate); the
      7 window offsets load into PE registers with ONE TensorLoad; three
      windows pack per 2KB PSUM bank.
    - GpSimd: assemble windowed D2 tiles [112,168] from broadcast APs.
    - DVE: fused (D2 <= t^2) * sim with accumulation, one
      scalar_tensor_tensor per (view, bank): free size 504/168.
    - Final: partition-reduce via ones-matmul -> [1,2] per-core output.
  Host finish: sum the 8 cores' masked sums, divide by host counts.
"""

import sys

sys.path.insert(0, "/opt/trn_rl_repo")

import numpy as np

import concourse.bass as bass
import concourse.mybir as mybir
import concourse.tile as tile
from concourse import bacc
from concourse.bass import broadcast_tensor_aps
from concourse.bass_utils import run_bass_kernel_spmd

B, C, H, W = 64, 256, 28, 28
N = H * W  # 784
NPAD = 800  # y-side padded so stationary slices are full 128 columns
NCORES = 8
BPC = B // NCORES  # batches per core
NT = 7  # n tiles: 7 groups of 4 image rows
P = 112  # partitions per tile (4 image rows)
THR = 0.7
WW = 6  # window rows (i') per n-tile (verified on host against inputs)
WWC = WW * 28  # 168 window columns in m

F32 = mybir.dt.float32
F16 = mybir.dt.float16
I32 = mybir.dt.int32
ALU = mybir.AluOpType
ENG = mybir.EngineType

_COMPILED = {}


def _build_nc():
    nc = bacc.Bacc("TRN2", debug=False, num_devices=NCORES)

    ins = {
        "ay1": nc.dram_tensor("ay1", [BPC, 128, 2, NPAD], F16, kind="ExternalInput"),
        "ay2": nc.dram_tensor("ay2", [BPC, 128, 2, NPAD], F16, kind="ExternalInput"),
        "bz2": nc.dram_tensor("bz2", [BPC, 128, 2, N], F16, kind="ExternalInput"),
        "bz1": nc.dram_tensor("bz1", [BPC, 128, 2, N], F16, kind="ExternalInput"),
    }
    dyw_i = nc.dram_tensor("dyw", [BPC, P, NT, WW], F32, kind="ExternalInput")
    dx_i = nc.dram_tensor("dx", [BPC, P, 28], F32, kind="ExternalInput")
    thr_i = nc.dram_tensor("thr", [BPC, P, 2], F32, kind="ExternalInput")
    woff_i = nc.dram_tensor("woff", [BPC, 1, NT], I32, kind="ExternalInput")
    out = nc.dram_tensor("out", [1, 2], F32, kind="ExternalOutput")

    with tile.TileContext(nc) as tc:
        with (
            tc.tile_pool(name="feat", bufs=2) as feat_pool,
            tc.tile_pool(name="dyx", bufs=2) as dyx_pool,
            tc.tile_pool(name="d2", bufs=3) as d2_pool,
            tc.tile_pool(name="scr", bufs=3) as scr_pool,
            tc.tile_pool(name="small", bufs=2) as sm_pool,
            tc.tile_pool(name="accum", bufs=1) as acc_pool,
            tc.tile_pool(name="pt", bufs=5, space="PSUM") as pt_pool,
            tc.tile_pool(name="ps", bufs=2, space="PSUM") as ps_pool,
            tc.tile_pool(name="psumf", bufs=1, space="PSUM") as psumf_pool,
        ):
            # stot[:, v, b*3+t] accumulates the masked sim sum of bank-group
            # t of batch b, view v
            stot = acc_pool.tile([P, 2, BPC * 3], F32)
            ones_col = acc_pool.tile([P, 1], F32)
            nc.vector.memset(ones_col[:, :], 1.0)

            for b in range(BPC):
                feats = {}
                for nm, eng in (
                    ("ay1", nc.sync),
                    ("bz2", nc.scalar),
                    ("ay2", nc.sync),
                    ("bz1", nc.scalar),
                ):
                    cols = NPAD if nm.startswith("ay") else N
                    t = feat_pool.tile([128, 2, cols], F16, tag=nm)
                    eng.dma_start(t[:, :, :], ins[nm][b])
                    feats[nm] = t
                dyw_t = dyx_pool.tile([P, NT, WW], F32, tag="dy")
                nc.gpsimd.dma_start(dyw_t[:, :, :], dyw_i[b])
                dx_t = dyx_pool.tile([P, 28], F32, tag="dx")
                nc.gpsimd.dma_start(dx_t[:, :], dx_i[b])
                thr_t = sm_pool.tile([P, 2], F32, tag="thr")
                nc.gpsimd.dma_start(thr_t[:, :], thr_i[b])
                woff_t = sm_pool.tile([1, NT], I32, tag="woff")
                nc.gpsimd.dma_start(woff_t[:, :], woff_i[b])

                regs = [
                    nc.alloc_register(ENG.PE, f"w_{b}_{k}") for k in range(NT)
                ]
                nc.tensor.load(regs, woff_t[0:1, 0:NT])
                wvs = [
                    nc.snap(reg, donate=True, min_val=0, max_val=(28 - WW) * 28)
                    for reg in regs
                ]

                # bank-groups: k in {0,1,2}, {3,4,5} pack 3 windows per PSUM
                # bank; k=6 gets its own
                for g in range(3):
                    ks = (g * 3, g * 3 + 1, g * 3 + 2) if g < 2 else (6,)
                    if g < 2:
                        d2t = d2_pool.tile([P, 3, WWC], F32, tag="d2")
                        nums = [
                            pt_pool.tile(
                                [128, 3, WWC], F32, tag="pt", name=f"pt_{b}_{g}_{v}"
                            )
                            for v in (0, 1)
                        ]
                    else:
                        d2t = d2_pool.tile([P, 1, WWC], F32, tag="d2s")
                        nums = [
                            ps_pool.tile(
                                [128, 1, WWC], F32, tag="ps", name=f"ps_{b}_{g}_{v}"
                            )
                            for v in (0, 1)
                        ]
                    for j, k in enumerate(ks):
                        i0, i1 = broadcast_tensor_aps(
                            dyw_t[:, k, :, None], dx_t[:, None, :]
                        )
                        nc.gpsimd.tensor_tensor(
                            d2t[:, j, :].rearrange("q (a c) -> q a c", a=WW),
                            i0,
                            i1,
                            ALU.add,
                        )
                        for v, (a_nm, b_nm) in enumerate(
                            (("ay1", "bz2"), ("ay2", "bz1"))
                        ):
                            for cc in (0, 1):
                                nc.tensor.matmul(
                                    nums[v][:, j, :],
                                    feats[a_nm][:, cc, k * P : k * P + 128],
                                    feats[b_nm][:, cc, bass.ds(wvs[k], WWC)],
                                    start=(cc == 0),
                                    stop=(cc == 1),
                                )
                    nw = len(ks)
                    for v in (0, 1):
                        scr = scr_pool.tile([P, 3 * WWC], F32, tag="scr")
                        nc.vector.scalar_tensor_tensor(
                            out=scr[:, 0 : nw * WWC],
                            in0=d2t[:, :, :],
                            scalar=thr_t[:, v : v + 1],
                            in1=nums[v][0:P, :, :],
                            op0=ALU.is_le,
                            op1=ALU.mult,
                            accum_out=stot[:, v, 3 * b + g : 3 * b + g + 1],
                        )

            sfin = acc_pool.tile([P, 2], F32)
            nc.vector.reduce_sum(sfin[:, :], stot[:, :, :], axis=mybir.AxisListType.X)
            ps_f = psumf_pool.tile([1, 2], F32)
            nc.tensor.matmul(
                ps_f[:, :], ones_col[:, :], sfin[:, :], start=True, stop=True
            )
            out_s = acc_pool.tile([1, 2], F32)
            nc.vector.tensor_copy(out_s[:, :], ps_f[:, :])
            nc.sync.dma_start(out[:, :], out_s[:, :])

    nc.compile()
    return nc


def _get_nc():
    if "nc" not in _COMPILED:
        _COMPILED["nc"] = _build_nc()
    return _COMPILED["nc"]


def _prep_host(y1, y2, z1, z2, view1_grid, view2_grid):
    """Host-side prep: separable distance tables, norms, counts, shards."""
    y1f = y1.reshape(B, C, N)
    y2f = y2.reshape(B, C, N)
    z1f = z1.reshape(B, C, N)
    z2f = z2.reshape(B, C, N)

    # --- separable grid tables ------------------------------------------
    g1y = view1_grid[:, 0, :, 0]  # [B, 28] rows (y coordinate per i)
    g1x = view1_grid[:, 1, 0, :]  # [B, 28] cols (x coordinate per j)
    g2y = view2_grid[:, 0, :, 0]
    g2x = view2_grid[:, 1, 0, :]
    if not (
        np.array_equal(view1_grid[:, 0], np.broadcast_to(g1y[:, :, None], (B, H, W)))
        and np.array_equal(view1_grid[:, 1], np.broadcast_to(g1x[:, None, :], (B, H, W)))
        and np.array_equal(view2_grid[:, 0], np.broadcast_to(g2y[:, :, None], (B, H, W)))
        and np.array_equal(view2_grid[:, 1], np.broadcast_to(g2x[:, None, :], (B, H, W)))
    ):
        raise RuntimeError("grids are not separable; unsupported input")

    dy = g1y[:, :, None] - g2y[:, None, :]  # fp32 [B,28,28]
    dx = g1x[:, :, None] - g2x[:, None, :]
    dy2 = dy * dy
    dx2 = dx * dx

    v1bin = np.linalg.norm(
        view1_grid[..., 1, 1] - view1_grid[..., 0, 0], axis=-1
    )  # [B]
    v2bin = np.linalg.norm(view2_grid[..., 1, 1] - view2_grid[..., 0, 0], axis=-1)
    t2 = np.empty((B, 2), np.float32)
    t2[:, 0] = ((THR * v1bin.astype(np.float64)) ** 2).astype(np.float32)
    t2[:, 1] = ((THR * v2bin.astype(np.float64)) ** 2).astype(np.float32)

    # --- per-(batch, tile) windows of valid i' --------------------------
    # A masked pair has dy2 <= d2/(1-2^-24) <= t2*(1+1.2e-7) < tmax2, so the
    # [first,last] band below covers every masked i'.
    tmax2 = np.maximum(t2[:, 0], t2[:, 1]).astype(np.float64) * (1 + 1e-6)  # [B]
    w0 = np.zeros((B, NT), np.int32)
    dyw = np.zeros((B, P, NT, WW), np.float32)
    iidx = np.arange(P) // 28  # [112] image row within tile
    for k in range(NT):
        sub_min = dy2[:, 4 * k : 4 * k + 4, :].min(axis=1)  # [B, 28]
        valid = sub_min <= tmax2[:, None]  # [B, 28]
        any_valid = valid.any(axis=1)
        first = np.argmax(valid, axis=1)
        last = 27 - np.argmax(valid[:, ::-1], axis=1)
        width = np.where(any_valid, last - first + 1, 1)
        if (width > WW).any():
            raise RuntimeError("mask window exceeds WW; unsupported input")
        w0k = np.minimum(np.where(any_valid, first, 0), 28 - WW).astype(np.int32)
        w0[:, k] = w0k
        cols = w0k[:, None] + np.arange(WW)[None, :]  # [B, WW]
        dyw[:, :, k, :] = dy2[
            np.arange(B)[:, None, None],
            (4 * k + iidx)[None, :, None],
            cols[:, None, :],
        ]
    woff = (w0 * 28).astype(np.int32).reshape(B, 1, NT)

    dxt = np.ascontiguousarray(
        np.broadcast_to(dx2[:, None, :, :], (B, 4, 28, 28)).reshape(B, P, 28)
    )

    # --- mask counts (bit-identical fp32 add + compare as device) -------
    counts = np.zeros(2, np.int64)
    for b in range(B):
        d2b = dy2[b][:, None, :, None] + dx2[b][None, :, None, :]  # fp32
        counts[0] += int((d2b <= t2[b, 0]).sum())
        counts[1] += int((d2b <= t2[b, 1]).sum())

    # --- norms ----------------------------------------------------------
    def rnorm(a):
        n = np.sqrt(np.einsum("bcn,bcn->bn", a, a, dtype=np.float32))
        return 1.0 / np.maximum(n, np.float32(1e-7))

    def pack(a, rn, cols):
        # [B, C, N] fp32 * per-col 1/norm -> [B, 128, 2, cols] f16
        ah = (a * rn[:, None, :]).reshape(B, 2, 128, N).transpose(0, 2, 1, 3)
        outp = np.zeros((B, 128, 2, cols), np.float16)
        outp[:, :, :, :N] = ah.astype(np.float16)
        return outp

    ay1 = pack(y1f, rnorm(y1f), NPAD)
    ay2 = pack(y2f, rnorm(y2f), NPAD)
    bz2 = pack(z2f, rnorm(z2f), N)
    bz1 = pack(z1f, rnorm(z1f), N)

    thr = np.ascontiguousarray(np.broadcast_to(t2[:, None, :], (B, P, 2)))

    in_maps = []
    for c in range(NCORES):
        s = slice(c * BPC, (c + 1) * BPC)
        in_maps.append(
            {
                "ay1": ay1[s],
                "ay2": ay2[s],
                "bz2": bz2[s],
                "bz1": bz1[s],
                "dyw": np.ascontiguousarray(dyw[s]),
                "dx": dxt[s],
                "thr": thr[s],
                "woff": np.ascontiguousarray(woff[s]),
            }
        )
    return in_maps, counts


def kernel(y1, y2, z1, z2, view1_grid, view2_grid):
    y1 = np.asarray(y1, np.float32)
    y2 = np.asarray(y2, np.float32)
    z1 = np.asarray(z1, np.float32)
    z2 = np.asarray(z2, np.float32)
    view1_grid = np.asarray(view1_grid, np.float32)
    view2_grid = np.asarray(view2_grid, np.float32)

    in_maps, counts = _prep_host(y1, y2, z1, z2, view1_grid, view2_grid)
    nc = _get_nc()
    res = run_bass_kernel_spmd(nc, in_maps, core_ids=list(range(NCORES)))
    s = np.zeros(2, np.float64)
    for i in range(NCORES):
        s += res.results[i]["out"][0].astype(np.float64)
    loss = -(
        np.float32(s[0]) / np.float32(counts[0])
        + np.float32(s[1]) / np.float32(counts[1])
    )
    return np.array(loss, dtype=np.float32)


# revision 12
# speedup vs baseline: 1.8548x; 1.1420x over previous
"""ConsistencyLoss kernel for 8 Trainium2 NeuronCores.

Math (per reference):
  For view1: sim = cos_sim_pairwise(y1, z2) [B,N,N]; mask from grid distances;
  loss_v = sum(sim*mask)/sum(mask); out = -(loss_1 + loss_2), N = 28*28 = 784.

Strategy: data-parallel over batch (8 batches/core x 8 cores).
  Host prep (cheap O(B*C*N) numpy):
    - The reference grids are separable: grid[b,0,i,j] depends only on i,
      grid[b,1,i,j] only on j.  Pairwise squared distance
      D2[n,m] = Dy2[i(n),i'(m)] + Dx2[j(n),j'(m)] from two [28,28] tables.
    - n is tiled in 7 groups of 4 image rows (112 partitions, aligned to the
      28-col image width).  For each tile the masked i' band spans at most
      WW=6 image rows whose start the host computes; the device evaluates
      only the [112, 168] window instead of [112, 784].
    - BOTH feature sides are normalized on host (fp32) and shipped as f16,
      so the windowed matmul directly produces cosine sims and the masked
      sums accumulate freely across tiles.  y-side is padded to 800 cols so
      every stationary load is a full 128 columns (enables FWL).
    - Mask counts (denominators) are computed on host with bit-identical
      fp32 arithmetic to the device mask test.
  Device per batch:
    - PE: sim = y_hat^T @ z_hat windowed (f16, fp32 PSUM accumulate); the
      7 window offsets load into PE registers with ONE TensorLoad; three
      windows pack per 2KB PSUM bank.
    - GpSimd: assemble windowed D2 tiles [112,168] from broadcast APs.
    - DVE: fused (D2 <= t^2) * sim with accumulation, one
      scalar_tensor_tensor per (view, bank): free size 504/168.
    - Final: partition-reduce via ones-matmul -> [1,2] per-core output.
  Host finish: sum the 8 cores' masked sums, divide by host counts.
"""

import sys

sys.path.insert(0, "/opt/trn_rl_repo")

import numpy as np

import concourse.bass as bass
import concourse.mybir as mybir
import concourse.tile as tile
from concourse import bacc
from concourse.bass import broadcast_tensor_aps
from concourse.bass_utils import run_bass_kernel_spmd

B, C, H, W = 64, 256, 28, 28
N = H * W  # 784
NPAD = 800  # y-side padded so stationary slices are full 128 columns
NCORES = 8
BPC = B // NCORES  # batches per core
NT = 7  # n tiles: 7 groups of 4 image rows
P = 112  # partitions per tile (4 image rows)
THR = 0.7
WW = 6  # window rows (i') per n-tile (verified on host against inputs)
WWC = WW * 28  # 168 window columns in m

F32 = mybir.dt.float32
F16 = mybir.dt.float16
I32 = mybir.dt.int32
ALU = mybir.AluOpType
ENG = mybir.EngineType

_COMPILED = {}


def _build_nc():
    nc = bacc.Bacc("TRN2", debug=False, num_devices=NCORES)

    ins = {
        "ay1": nc.dram_tensor("ay1", [BPC, 128, 2, NPAD], F16, kind="ExternalInput"),
        "ay2": nc.dram_tensor("ay2", [BPC, 128, 2, NPAD], F16, kind="ExternalInput"),
        "bz2": nc.dram_tensor("bz2", [BPC, 128, 2, N], F16, kind="ExternalInput"),
        "bz1": nc.dram_tensor("bz1", [BPC, 128, 2, N], F16, kind="ExternalInput"),
    }
    # per-batch 72 fp32 cols: 0:42 dyw [NT,WW], 42:70 dx2 row, 70:72 thr
    tbl_i = nc.dram_tensor("tbl", [P, BPC, 72], F32, kind="ExternalInput")
    woff_i = nc.dram_tensor("woff", [1, BPC * NT], I32, kind="ExternalInput")
    out = nc.dram_tensor("out", [1, 2], F32, kind="ExternalOutput")

    with tile.TileContext(nc) as tc:
        with (
            tc.tile_pool(name="feat", bufs=2) as feat_pool,
            tc.tile_pool(name="d2", bufs=3) as d2_pool,
            tc.tile_pool(name="scr", bufs=3) as scr_pool,
            tc.tile_pool(name="accum", bufs=1) as acc_pool,
            tc.tile_pool(name="pt", bufs=5, space="PSUM") as pt_pool,
            tc.tile_pool(name="ps", bufs=2, space="PSUM") as ps_pool,
            tc.tile_pool(name="psumf", bufs=1, space="PSUM") as psumf_pool,
        ):
            # stot[:, v, b*3+t] accumulates the masked sim sum of bank-group
            # t of batch b, view v
            stot = acc_pool.tile([P, 2, BPC * 3], F32)
            ones_col = acc_pool.tile([P, 1], F32)
            nc.vector.memset(ones_col[:, :], 1.0)

            tb = acc_pool.tile([P, BPC, 72], F32)
            nc.sync.dma_start(tb[:, :, :], tbl_i[:, :, :])
            woff_t = acc_pool.tile([1, BPC * NT], I32)
            nc.sync.dma_start(woff_t[:, :], woff_i[:, :])
            wvs_all = []

            def load_offsets(lo, hi):
                regs = [
                    nc.alloc_register(ENG.PE, f"w_{i}") for i in range(lo, hi)
                ]
                nc.tensor.load(regs, woff_t[0:1, lo:hi])
                wvs_all.extend(
                    nc.snap(reg, donate=True, min_val=0, max_val=(28 - WW) * 28)
                    for reg in regs
                )

            load_offsets(0, BPC * NT // 2)

            for b in range(BPC):
                if b == BPC // 2:
                    load_offsets(BPC * NT // 2, BPC * NT)
                feats = {}
                for nm, eng in (
                    ("ay1", nc.sync),
                    ("bz2", nc.scalar),
                    ("ay2", nc.sync),
                    ("bz1", nc.scalar),
                ):
                    cols = NPAD if nm.startswith("ay") else N
                    t = feat_pool.tile([128, 2, cols], F16, tag=nm)
                    eng.dma_start(t[:, :, :], ins[nm][b])
                    feats[nm] = t
                wvs = wvs_all[b * NT : (b + 1) * NT]

                # bank-groups: k in {0,1,2}, {3,4,5} pack 3 windows per PSUM
                # bank; k=6 gets its own
                for g in range(3):
                    ks = (g * 3, g * 3 + 1, g * 3 + 2) if g < 2 else (6,)
                    if g < 2:
                        d2t = d2_pool.tile([P, 3, WWC], F32, tag="d2")
                        nums = [
                            pt_pool.tile(
                                [128, 3, WWC], F32, tag="pt", name=f"pt_{b}_{g}_{v}"
                            )
                            for v in (0, 1)
                        ]
                    else:
                        d2t = d2_pool.tile([P, 1, WWC], F32, tag="d2s")
                        nums = [
                            ps_pool.tile(
                                [128, 1, WWC], F32, tag="ps", name=f"ps_{b}_{g}_{v}"
                            )
                            for v in (0, 1)
                        ]
                    for j, k in enumerate(ks):
                        i0, i1 = broadcast_tensor_aps(
                            tb[:, b, 6 * k : 6 * k + 6, None],
                            tb[:, b, None, 42:70],
                        )
                        nc.gpsimd.tensor_tensor(
                            d2t[:, j, :].rearrange("q (a c) -> q a c", a=WW),
                            i0,
                            i1,
                            ALU.add,
                        )
                        for v, (a_nm, b_nm) in enumerate(
                            (("ay1", "bz2"), ("ay2", "bz1"))
                        ):
                            for cc in (0, 1):
                                nc.tensor.matmul(
                                    nums[v][:, j, :],
                                    feats[a_nm][:, cc, k * P : k * P + 128],
                                    feats[b_nm][:, cc, bass.ds(wvs[k], WWC)],
                                    start=(cc == 0),
                                    stop=(cc == 1),
                                )
                    nw = len(ks)
                    for v in (0, 1):
                        scr = scr_pool.tile([P, 3 * WWC], F32, tag="scr")
                        nc.vector.scalar_tensor_tensor(
                            out=scr[:, 0 : nw * WWC],
                            in0=d2t[:, :, :],
                            scalar=tb[:, b, 70 + v : 71 + v],
                            in1=nums[v][0:P, :, :],
                            op0=ALU.is_le,
                            op1=ALU.mult,
                            accum_out=stot[:, v, 3 * b + g : 3 * b + g + 1],
                        )

            sfin = acc_pool.tile([P, 2], F32)
            nc.vector.reduce_sum(sfin[:, :], stot[:, :, :], axis=mybir.AxisListType.X)
            ps_f = psumf_pool.tile([1, 2], F32)
            nc.tensor.matmul(
                ps_f[:, :], ones_col[:, :], sfin[:, :], start=True, stop=True
            )
            out_s = acc_pool.tile([1, 2], F32)
            nc.vector.tensor_copy(out_s[:, :], ps_f[:, :])
            nc.sync.dma_start(out[:, :], out_s[:, :])

    nc.compile()
    return nc


def _get_nc():
    if "nc" not in _COMPILED:
        _COMPILED["nc"] = _build_nc()
    return _COMPILED["nc"]


def _prep_host(y1, y2, z1, z2, view1_grid, view2_grid):
    """Host-side prep: separable distance tables, norms, counts, shards."""
    y1f = y1.reshape(B, C, N)
    y2f = y2.reshape(B, C, N)
    z1f = z1.reshape(B, C, N)
    z2f = z2.reshape(B, C, N)

    # --- separable grid tables ------------------------------------------
    g1y = view1_grid[:, 0, :, 0]  # [B, 28] rows (y coordinate per i)
    g1x = view1_grid[:, 1, 0, :]  # [B, 28] cols (x coordinate per j)
    g2y = view2_grid[:, 0, :, 0]
    g2x = view2_grid[:, 1, 0, :]
    if not (
        np.array_equal(view1_grid[:, 0], np.broadcast_to(g1y[:, :, None], (B, H, W)))
        and np.array_equal(view1_grid[:, 1], np.broadcast_to(g1x[:, None, :], (B, H, W)))
        and np.array_equal(view2_grid[:, 0], np.broadcast_to(g2y[:, :, None], (B, H, W)))
        and np.array_equal(view2_grid[:, 1], np.broadcast_to(g2x[:, None, :], (B, H, W)))
    ):
        raise RuntimeError("grids are not separable; unsupported input")

    dy = g1y[:, :, None] - g2y[:, None, :]  # fp32 [B,28,28]
    dx = g1x[:, :, None] - g2x[:, None, :]
    dy2 = dy * dy
    dx2 = dx * dx

    v1bin = np.linalg.norm(
        view1_grid[..., 1, 1] - view1_grid[..., 0, 0], axis=-1
    )  # [B]
    v2bin = np.linalg.norm(view2_grid[..., 1, 1] - view2_grid[..., 0, 0], axis=-1)
    t2 = np.empty((B, 2), np.float32)
    t2[:, 0] = ((THR * v1bin.astype(np.float64)) ** 2).astype(np.float32)
    t2[:, 1] = ((THR * v2bin.astype(np.float64)) ** 2).astype(np.float32)

    # --- per-(batch, tile) windows of valid i' --------------------------
    # A masked pair has dy2 <= d2/(1-2^-24) <= t2*(1+1.2e-7) < tmax2, so the
    # [first,last] band below covers every masked i'.
    tmax2 = np.maximum(t2[:, 0], t2[:, 1]).astype(np.float64) * (1 + 1e-6)  # [B]
    w0 = np.zeros((B, NT), np.int32)
    dyw = np.zeros((B, P, NT, WW), np.float32)
    iidx = np.arange(P) // 28  # [112] image row within tile
    for k in range(NT):
        sub_min = dy2[:, 4 * k : 4 * k + 4, :].min(axis=1)  # [B, 28]
        valid = sub_min <= tmax2[:, None]  # [B, 28]
        any_valid = valid.any(axis=1)
        first = np.argmax(valid, axis=1)
        last = 27 - np.argmax(valid[:, ::-1], axis=1)
        width = np.where(any_valid, last - first + 1, 1)
        if (width > WW).any():
            raise RuntimeError("mask window exceeds WW; unsupported input")
        w0k = np.minimum(np.where(any_valid, first, 0), 28 - WW).astype(np.int32)
        w0[:, k] = w0k
        cols = w0k[:, None] + np.arange(WW)[None, :]  # [B, WW]
        dyw[:, :, k, :] = dy2[
            np.arange(B)[:, None, None],
            (4 * k + iidx)[None, :, None],
            cols[:, None, :],
        ]
    woff = (w0 * 28).astype(np.int32)  # [B, NT]

    dxt = np.broadcast_to(dx2[:, None, :, :], (B, 4, 28, 28)).reshape(B, P, 28)

    # --- mask counts (bit-identical fp32 add + compare as device) -------
    counts = np.zeros(2, np.int64)
    for b in range(B):
        d2b = dy2[b][:, None, :, None] + dx2[b][None, :, None, :]  # fp32
        counts[0] += int((d2b <= t2[b, 0]).sum())
        counts[1] += int((d2b <= t2[b, 1]).sum())

    # --- norms ----------------------------------------------------------
    def rnorm(a):
        n = np.sqrt(np.einsum("bcn,bcn->bn", a, a, dtype=np.float32))
        return 1.0 / np.maximum(n, np.float32(1e-7))

    def pack(a, rn, cols):
        # [B, C, N] fp32 * per-col 1/norm -> [B, 128, 2, cols] f16
        ah = (a * rn[:, None, :]).reshape(B, 2, 128, N).transpose(0, 2, 1, 3)
        outp = np.zeros((B, 128, 2, cols), np.float16)
        outp[:, :, :, :N] = ah.astype(np.float16)
        return outp

    ay1 = pack(y1f, rnorm(y1f), NPAD)
    ay2 = pack(y2f, rnorm(y2f), NPAD)
    bz2 = pack(z2f, rnorm(z2f), N)
    bz1 = pack(z1f, rnorm(z1f), N)

    # merged per-batch table: [B, P, 72] -> core layout [P, BPC, 72]
    tbl = np.empty((B, P, 72), np.float32)
    tbl[:, :, 0:42] = dyw.reshape(B, P, NT * WW)
    tbl[:, :, 42:70] = dxt
    tbl[:, :, 70:72] = np.broadcast_to(t2[:, None, :], (B, P, 2))

    in_maps = []
    for c in range(NCORES):
        s = slice(c * BPC, (c + 1) * BPC)
        in_maps.append(
            {
                "ay1": ay1[s],
                "ay2": ay2[s],
                "bz2": bz2[s],
                "bz1": bz1[s],
                "tbl": np.ascontiguousarray(tbl[s].transpose(1, 0, 2)),
                "woff": np.ascontiguousarray(woff[s].reshape(1, BPC * NT)),
            }
        )
    return in_maps, counts


def kernel(y1, y2, z1, z2, view1_grid, view2_grid):
    y1 = np.asarray(y1, np.float32)
    y2 = np.asarray(y2, np.float32)
    z1 = np.asarray(z1, np.float32)
    z2 = np.asarray(z2, np.float32)
    view1_grid = np.asarray(view1_grid, np.float32)
    view2_grid = np.asarray(view2_grid, np.float32)

    in_maps, counts = _prep_host(y1, y2, z1, z2, view1_grid, view2_grid)
    nc = _get_nc()
    res = run_bass_kernel_spmd(nc, in_maps, core_ids=list(range(NCORES)))
    s = np.zeros(2, np.float64)
    for i in range(NCORES):
        s += res.results[i]["out"][0].astype(np.float64)
    loss = -(
        np.float32(s[0]) / np.float32(counts[0])
        + np.float32(s[1]) / np.float32(counts[1])
    )
    return np.array(loss, dtype=np.float32)


# revision 21
# speedup vs baseline: 1.9503x; 1.0515x over previous
"""ConsistencyLoss kernel for 8 Trainium2 NeuronCores.

Math (per reference):
  For view1: sim = cos_sim_pairwise(y1, z2) [B,N,N]; mask from grid distances;
  loss_v = sum(sim*mask)/sum(mask); out = -(loss_1 + loss_2), N = 28*28 = 784.

Strategy: data-parallel over batch (8 batches/core x 8 cores).
  Host prep (cheap O(B*C*N) numpy):
    - The reference grids are separable: grid[b,0,i,j] depends only on i,
      grid[b,1,i,j] only on j.  Pairwise squared distance
      D2[n,m] = Dy2[i(n),i'(m)] + Dx2[j(n),j'(m)] from two [28,28] tables.
    - n is tiled in 7 groups of 4 image rows (112 partitions, aligned to the
      28-col image width).  For each tile the masked i' band spans at most
      WW=6 image rows whose start the host computes; the device evaluates
      only the [112, 168] window instead of [112, 784].
    - BOTH feature sides are normalized on host (fp32) and shipped as f16,
      so the windowed matmul directly produces cosine sims and the masked
      sums accumulate freely across tiles.  y-side is padded to 800 cols so
      every stationary load is a full 128 columns (enables FWL).
    - Mask counts (denominators) are computed on host with bit-identical
      fp32 arithmetic to the device mask test.
  Device per batch:
    - PE: sim = y_hat^T @ z_hat windowed (f16, fp32 PSUM accumulate); the
      7 window offsets load into PE registers with ONE TensorLoad; three
      windows pack per 2KB PSUM bank.
    - GpSimd: assemble windowed D2 tiles [112,168] from broadcast APs.
    - DVE: fused (D2 <= t^2) * sim with accumulation, one
      scalar_tensor_tensor per (view, bank): free size 504/168.
    - Final: partition-reduce via ones-matmul -> [1,2] per-core output.
  Host finish: sum the 8 cores' masked sums, divide by host counts.
"""

import sys

sys.path.insert(0, "/opt/trn_rl_repo")

import numpy as np

import concourse.bass as bass
import concourse.mybir as mybir
import concourse.tile as tile
from concourse import bacc
from concourse.bass import broadcast_tensor_aps
from concourse.bass_utils import run_bass_kernel_spmd

B, C, H, W = 64, 256, 28, 28
N = H * W  # 784
NPAD = 800  # y-side padded so stationary slices are full 128 columns
NCORES = 8
BPC = B // NCORES  # batches per core
NT = 7  # n tiles: 7 groups of 4 image rows
P = 112  # partitions per tile (4 image rows)
THR = 0.7
WW = 6  # window rows (i') per n-tile (verified on host against inputs)
WWC = WW * 28  # 168 window columns in m

F32 = mybir.dt.float32
F16 = mybir.dt.float16
I32 = mybir.dt.int32
ALU = mybir.AluOpType
ENG = mybir.EngineType

_COMPILED = {}


def _build_nc():
    nc = bacc.Bacc("TRN2", debug=False, num_devices=NCORES)

    # features grouped in batch-pairs so each DMA moves 2 batches (bigger
    # per-partition packets -> better DMA engine throughput)
    ins = {
        "ay1": nc.dram_tensor(
            "ay1", [BPC // 2, 128, 2, 2, NPAD], F16, kind="ExternalInput"
        ),
        "ay2": nc.dram_tensor(
            "ay2", [BPC // 2, 128, 2, 2, NPAD], F16, kind="ExternalInput"
        ),
        "bz2": nc.dram_tensor(
            "bz2", [BPC // 2, 128, 2, 2, N], F16, kind="ExternalInput"
        ),
        "bz1": nc.dram_tensor(
            "bz1", [BPC // 2, 128, 2, 2, N], F16, kind="ExternalInput"
        ),
    }
    # per-batch 72 fp32 cols: 0:42 dyw [NT,WW], 42:70 dx2 row, 70:72 thr
    tbl_i = nc.dram_tensor("tbl", [P, BPC, 72], F32, kind="ExternalInput")
    woff_i = nc.dram_tensor("woff", [1, BPC * NT], I32, kind="ExternalInput")
    out = nc.dram_tensor("out", [1, 2], F32, kind="ExternalOutput")

    with tile.TileContext(nc) as tc:
        with (
            tc.tile_pool(name="feat", bufs=2) as feat_pool,
            tc.tile_pool(name="d2", bufs=3) as d2_pool,
            tc.tile_pool(name="scr", bufs=3) as scr_pool,
            tc.tile_pool(name="accum", bufs=1) as acc_pool,
            tc.tile_pool(name="pt", bufs=7, space="PSUM") as pt_pool,
            tc.tile_pool(name="psumf", bufs=1, space="PSUM") as psumf_pool,
        ):
            # stot[:, v, b*3+t] accumulates the masked sim sum of bank-group
            # t of batch b, view v
            stot = acc_pool.tile([P, 2, BPC * 3], F32)
            ones_col = acc_pool.tile([P, 1], F32)
            nc.vector.memset(ones_col[:, :], 1.0)

            tb = acc_pool.tile([P, BPC, 72], F32)
            nc.scalar.dma_start(tb[:, :, :], tbl_i[:, :, :])
            woff_t = acc_pool.tile([1, BPC * NT], I32)
            nc.scalar.dma_start(woff_t[:, :], woff_i[:, :])
            wvs_all = []

            def load_offsets(lo, hi):
                regs = [
                    nc.alloc_register(ENG.PE, f"w_{i}") for i in range(lo, hi)
                ]
                nc.tensor.load(regs, woff_t[0:1, lo:hi])
                wvs_all.extend(
                    nc.snap(reg, donate=True, min_val=0, max_val=(28 - WW) * 28)
                    for reg in regs
                )

            load_offsets(0, BPC * NT // 2)

            pair_feats = {}
            for b in range(BPC):
                if b == BPC // 2:
                    load_offsets(BPC * NT // 2, BPC * NT)
                if b % 2 == 0:
                    for nm, eng in (
                        ("ay1", nc.sync),
                        ("bz2", nc.scalar),
                        ("ay2", nc.sync),
                        ("bz1", nc.scalar),
                    ):
                        cols = NPAD if nm.startswith("ay") else N
                        t = feat_pool.tile([128, 2, 2, cols], F16, tag=nm)
                        eng.dma_start(t[:, :, :, :], ins[nm][b // 2])
                        pair_feats[nm] = t
                sub = b % 2
                feats = {nm: t[:, sub] for nm, t in pair_feats.items()}
                wvs = wvs_all[b * NT : (b + 1) * NT]

                # bank-groups: k in {0,1,2}, {3,4,5} pack 3 windows per PSUM
                # bank; k=6 gets its own
                for g in range(3):
                    ks = (g * 3, g * 3 + 1, g * 3 + 2) if g < 2 else (6,)
                    if g < 2:
                        d2t = d2_pool.tile([P, 3, WWC], F32, tag="d2")
                    else:
                        d2t = d2_pool.tile([P, 1, WWC], F32, tag="d2s")
                    nums = [
                        pt_pool.tile(
                            [128, 3, WWC], F32, tag="pt", name=f"pt_{b}_{g}_{v}"
                        )
                        for v in (0, 1)
                    ]
                    for j, k in enumerate(ks):
                        i0, i1 = broadcast_tensor_aps(
                            tb[:, b, 6 * k : 6 * k + 6, None],
                            tb[:, b, None, 42:70],
                        )
                        nc.gpsimd.tensor_tensor(
                            d2t[:, j, :].rearrange("q (a c) -> q a c", a=WW),
                            i0,
                            i1,
                            ALU.add,
                        )
                        for v, (a_nm, b_nm) in enumerate(
                            (("ay1", "bz2"), ("ay2", "bz1"))
                        ):
                            for cc in (0, 1):
                                nc.tensor.matmul(
                                    nums[v][:, j, :],
                                    feats[a_nm][:, cc, k * P : k * P + 128],
                                    feats[b_nm][:, cc, bass.ds(wvs[k], WWC)],
                                    start=(cc == 0),
                                    stop=(cc == 1),
                                )
                    nw = len(ks)
                    for v in (0, 1):
                        scr = scr_pool.tile([P, 3 * WWC], F32, tag="scr")
                        nc.vector.scalar_tensor_tensor(
                            out=scr[:, 0 : nw * WWC],
                            in0=d2t[:, :, :],
                            scalar=tb[:, b, 70 + v : 71 + v],
                            in1=nums[v][0:P, 0:nw, :],
                            op0=ALU.is_le,
                            op1=ALU.mult,
                            accum_out=stot[:, v, 3 * b + g : 3 * b + g + 1],
                        )

            sfin = acc_pool.tile([P, 2], F32)
            nc.vector.reduce_sum(sfin[:, :], stot[:, :, :], axis=mybir.AxisListType.X)
            ps_f = psumf_pool.tile([1, 2], F32)
            nc.tensor.matmul(
                ps_f[:, :], ones_col[:, :], sfin[:, :], start=True, stop=True
            )
            out_s = acc_pool.tile([1, 2], F32)
            nc.vector.tensor_copy(out_s[:, :], ps_f[:, :])
            nc.sync.dma_start(out[:, :], out_s[:, :])

    nc.compile()
    return nc


def _get_nc():
    if "nc" not in _COMPILED:
        _COMPILED["nc"] = _build_nc()
    return _COMPILED["nc"]


def _prep_host(y1, y2, z1, z2, view1_grid, view2_grid):
    """Host-side prep: separable distance tables, norms, counts, shards."""
    y1f = y1.reshape(B, C, N)
    y2f = y2.reshape(B, C, N)
    z1f = z1.reshape(B, C, N)
    z2f = z2.reshape(B, C, N)

    # --- separable grid tables ------------------------------------------
    g1y = view1_grid[:, 0, :, 0]  # [B, 28] rows (y coordinate per i)
    g1x = view1_grid[:, 1, 0, :]  # [B, 28] cols (x coordinate per j)
    g2y = view2_grid[:, 0, :, 0]
    g2x = view2_grid[:, 1, 0, :]
    if not (
        np.array_equal(view1_grid[:, 0], np.broadcast_to(g1y[:, :, None], (B, H, W)))
        and np.array_equal(view1_grid[:, 1], np.broadcast_to(g1x[:, None, :], (B, H, W)))
        and np.array_equal(view2_grid[:, 0], np.broadcast_to(g2y[:, :, None], (B, H, W)))
        and np.array_equal(view2_grid[:, 1], np.broadcast_to(g2x[:, None, :], (B, H, W)))
    ):
        raise RuntimeError("grids are not separable; unsupported input")

    dy = g1y[:, :, None] - g2y[:, None, :]  # fp32 [B,28,28]
    dx = g1x[:, :, None] - g2x[:, None, :]
    dy2 = dy * dy
    dx2 = dx * dx

    v1bin = np.linalg.norm(
        view1_grid[..., 1, 1] - view1_grid[..., 0, 0], axis=-1
    )  # [B]
    v2bin = np.linalg.norm(view2_grid[..., 1, 1] - view2_grid[..., 0, 0], axis=-1)
    t2 = np.empty((B, 2), np.float32)
    t2[:, 0] = ((THR * v1bin.astype(np.float64)) ** 2).astype(np.float32)
    t2[:, 1] = ((THR * v2bin.astype(np.float64)) ** 2).astype(np.float32)

    # --- per-(batch, tile) windows of valid i' --------------------------
    # A masked pair has dy2 <= d2/(1-2^-24) <= t2*(1+1.2e-7) < tmax2, so the
    # [first,last] band below covers every masked i'.
    tmax2 = np.maximum(t2[:, 0], t2[:, 1]).astype(np.float64) * (1 + 1e-6)  # [B]
    w0 = np.zeros((B, NT), np.int32)
    dyw = np.zeros((B, P, NT, WW), np.float32)
    iidx = np.arange(P) // 28  # [112] image row within tile
    for k in range(NT):
        sub_min = dy2[:, 4 * k : 4 * k + 4, :].min(axis=1)  # [B, 28]
        valid = sub_min <= tmax2[:, None]  # [B, 28]
        any_valid = valid.any(axis=1)
        first = np.argmax(valid, axis=1)
        last = 27 - np.argmax(valid[:, ::-1], axis=1)
        width = np.where(any_valid, last - first + 1, 1)
        if (width > WW).any():
            raise RuntimeError("mask window exceeds WW; unsupported input")
        w0k = np.minimum(np.where(any_valid, first, 0), 28 - WW).astype(np.int32)
        w0[:, k] = w0k
        cols = w0k[:, None] + np.arange(WW)[None, :]  # [B, WW]
        dyw[:, :, k, :] = dy2[
            np.arange(B)[:, None, None],
            (4 * k + iidx)[None, :, None],
            cols[:, None, :],
        ]
    woff = (w0 * 28).astype(np.int32)  # [B, NT]

    dxt = np.broadcast_to(dx2[:, None, :, :], (B, 4, 28, 28)).reshape(B, P, 28)

    # --- mask counts (bit-identical fp32 add + compare as device) -------
    counts = np.zeros(2, np.int64)
    for b in range(B):
        d2b = dy2[b][:, None, :, None] + dx2[b][None, :, None, :]  # fp32
        counts[0] += int((d2b <= t2[b, 0]).sum())
        counts[1] += int((d2b <= t2[b, 1]).sum())

    # --- norms ----------------------------------------------------------
    def rnorm(a):
        n = np.sqrt(np.einsum("bcn,bcn->bn", a, a, dtype=np.float32))
        return 1.0 / np.maximum(n, np.float32(1e-7))

    def pack(a, rn, cols):
        # [B, C, N] fp32 * per-col 1/norm -> [B//2, 128, 2, 2, cols] f16
        # (batch pairs innermost-but-one so each DMA moves 2 batches)
        ah = (a * rn[:, None, :]).reshape(B, 2, 128, N).transpose(0, 2, 1, 3)
        outp = np.zeros((B, 128, 2, cols), np.float16)
        outp[:, :, :, :N] = ah.astype(np.float16)
        return np.ascontiguousarray(
            outp.reshape(B // 2, 2, 128, 2, cols).transpose(0, 2, 1, 3, 4)
        )

    ay1 = pack(y1f, rnorm(y1f), NPAD)
    ay2 = pack(y2f, rnorm(y2f), NPAD)
    bz2 = pack(z2f, rnorm(z2f), N)
    bz1 = pack(z1f, rnorm(z1f), N)

    # merged per-batch table: [B, P, 72] -> core layout [P, BPC, 72]
    tbl = np.empty((B, P, 72), np.float32)
    tbl[:, :, 0:42] = dyw.reshape(B, P, NT * WW)
    tbl[:, :, 42:70] = dxt
    tbl[:, :, 70:72] = np.broadcast_to(t2[:, None, :], (B, P, 2))

    in_maps = []
    for c in range(NCORES):
        s = slice(c * BPC, (c + 1) * BPC)
        sp = slice(c * BPC // 2, (c + 1) * BPC // 2)
        in_maps.append(
            {
                "ay1": ay1[sp],
                "ay2": ay2[sp],
                "bz2": bz2[sp],
                "bz1": bz1[sp],
                "tbl": np.ascontiguousarray(tbl[s].transpose(1, 0, 2)),
                "woff": np.ascontiguousarray(woff[s].reshape(1, BPC * NT)),
            }
        )
    return in_maps, counts


def kernel(y1, y2, z1, z2, view1_grid, view2_grid):
    y1 = np.asarray(y1, np.float32)
    y2 = np.asarray(y2, np.float32)
    z1 = np.asarray(z1, np.float32)
    z2 = np.asarray(z2, np.float32)
    view1_grid = np.asarray(view1_grid, np.float32)
    view2_grid = np.asarray(view2_grid, np.float32)

    in_maps, counts = _prep_host(y1, y2, z1, z2, view1_grid, view2_grid)
    nc = _get_nc()
    res = run_bass_kernel_spmd(nc, in_maps, core_ids=list(range(NCORES)))
    s = np.zeros(2, np.float64)
    for i in range(NCORES):
        s += res.results[i]["out"][0].astype(np.float64)
    loss = -(
        np.float32(s[0]) / np.float32(counts[0])
        + np.float32(s[1]) / np.float32(counts[1])
    )
    return np.array(loss, dtype=np.float32)
